# revision 38
# baseline (speedup 1.0000x reference)
"""Multi-head attention Trainium2 kernel.

Problem: B=4, L=2048, D=1024, H=16 heads (dk=dv=64).
Sharding: 8 cores = 4 batches x 2 head-groups. Core c handles batch c//2 and
heads (c%2)*8 .. (c%2)*8+8 (column-parallel W_q/W_k/W_v, row-parallel W_o).
Host sums each batch's two partial outputs and adds b_o.

Per-core dataflow (all matmuls bf16 with f32 PSUM accumulation):
  xT    = host-pretransposed x, bf16            [D, L]   (q, k, v inputs)
  qT_i  = W_q_chunk.T @ xT  (+b_q)              [128=2 heads, L]
  kT_h  = same for k, zero-padded per head      [128, L]  (K=128 keeps HAM warm)
  v     = xT_chunk.T @ W_v (natural layout)     [L, 8*65] with ones column
  S^T   = kT_h.T @ qT                           [Lk, Lq]  scores transposed
  P^T   = exp(S^T / 8)   (ScalarE, no max-sub: scores are bounded ~|2.5|)
  ctx~T = v~.T @ P^T     (v stationary)         [65, Lq]; row 64 = softmax denom
  ctx   = PE-transpose-back(ctx~T) * (1/denom)  normalized, per-partition scalars
  ctxT  = PE-transpose(ctx) + b_v               [8*64, L]
  out   = ctxT.T @ W_o   (natural [L, D])       partial f32, summed on host

Scheduling: the i=1..3 q/k projection groups, the normalization chains, and
the head-pair 0..2 ctx transposes are drip-fed into the ACT-bound attention
k-loop (one op per exp) so the PE uses the ScalarE slack; S(0) of each next
block is hoisted across the block boundary to keep the exp stream dense.
"""

import numpy as np
import ml_dtypes
from contextlib import ExitStack

import concourse.bacc as bacc
import concourse.mybir as mybir
import concourse.tile as tile
from concourse import bass_utils
from concourse.masks import make_identity

P = 128
B, L, D = 4, 2048, 1024
H, DK, DV = 16, 64, 64
HPC = 8            # heads per core
HCOLS = HPC * DK   # 512 projection columns per core
KC = D // P        # 8 contraction chunks
LC = L // P        # 16 L chunks
NB = 512           # matmul free-dim block (one PSUM bank of f32)
QB = 1024          # attention q-block
NQB = L // QB      # 2
VW = DV + 1        # 65: v plus ones column
BF16 = mybir.dt.bfloat16
F32 = mybir.dt.float32

_CACHE = {}


def build(dump=False):
    nc = bacc.Bacc("TRN2", target_bir_lowering=False, debug=False)

    # x inputs arrive pre-transposed from the host: [D, L] contiguous
    xq = nc.dram_tensor("xq", (D, L), BF16, kind="ExternalInput").ap()
    xk = nc.dram_tensor("xk", (D, L), BF16, kind="ExternalInput").ap()
    xv = nc.dram_tensor("xv", (D, L), BF16, kind="ExternalInput").ap()
    wq = nc.dram_tensor("wq", (D, HCOLS), BF16, kind="ExternalInput").ap()
    wk = nc.dram_tensor("wk", (D, HCOLS), BF16, kind="ExternalInput").ap()
    wv = nc.dram_tensor("wv", (D, HCOLS), BF16, kind="ExternalInput").ap()
    wo = nc.dram_tensor("wo", (HCOLS, D), BF16, kind="ExternalInput").ap()
    bq = nc.dram_tensor("bq", (P, HCOLS // P), F32, kind="ExternalInput").ap()
    bk = nc.dram_tensor("bk", (P, HCOLS // P), F32, kind="ExternalInput").ap()
    bv = nc.dram_tensor("bv", (P, HCOLS // P), F32, kind="ExternalInput").ap()
    out = nc.dram_tensor("out", (L, D), F32, kind="ExternalOutput").ap()
    if dump:
        d_qt = nc.dram_tensor("d_qt", (4, P, L), BF16, kind="ExternalOutput").ap()
        d_kt = nc.dram_tensor("d_kt", (HPC, P, L), BF16, kind="ExternalOutput").ap()
        d_vt = nc.dram_tensor("d_vt", (LC, P, HPC * VW), BF16, kind="ExternalOutput").ap()
        d_ctx = nc.dram_tensor("d_ctx", (LC, P, HCOLS), BF16, kind="ExternalOutput").ap()
        d_ctxT = nc.dram_tensor("d_ctxT", (4, P, L), BF16, kind="ExternalOutput").ap()

    EXP = mybir.ActivationFunctionType.Exp

    with tile.TileContext(nc) as tc, ExitStack() as ctx:
        persist = ctx.enter_context(tc.tile_pool(name="persist", bufs=1))

        # ---- persistent tiles ----
        # qt tile i holds heads 2i (rows 0:64) and 2i+1 (rows 64:128).
        qt = [persist.tile([P, L], BF16, tag=f"qt{i}", name=f"qt{i}") for i in range(4)]
        # kt is stored per head, zero-padded to the full 128 partitions so the
        # S^T matmul contracts K=128: head 2i lives in rows 0:64 (rows 64:128
        # zero), head 2i+1 in rows 64:128 (rows 0:64 zero). Both heads then
        # share the same full-height qt tile as the moving operand, and K=128
        # keeps the PE HAM activity monitor warm (K=64 matmuls don't register
        # as PE-busy and leave the clock throttled at 1.2 GHz).
        kt = [persist.tile([P, L], BF16, tag=f"kt{h}", name=f"kt{h}") for h in range(HPC)]
        # v~ tiles: [128 k-positions, 8 heads * 65]; col h*65+64 = 1.0
        vt = [persist.tile([P, HPC * VW], BF16, tag=f"vt{i}", name=f"vt{i}") for i in range(LC)]
        # ctx natural [L, 512] as 16 row tiles
        ctx_t = [persist.tile([P, HCOLS], BF16, tag=f"ctx{i}", name=f"ctx{i}") for i in range(LC)]
        bq_sb = persist.tile([P, HCOLS // P], F32, tag="bq", name="bq_sb")
        bk_sb = persist.tile([P, HCOLS // P], F32, tag="bk", name="bk_sb")
        bv_sb = persist.tile([P, HCOLS // P], F32, tag="bv", name="bv_sb")
        identity = persist.tile([P, P], BF16, tag="ident", name="ident")
        identity_f = persist.tile([P, P], F32, tag="identf", name="identf")

        # ---- projection-phase pool (lives until the last sprinkled
        # projection group inside the attention loop finishes) ----
        pctx = ExitStack()
        xt_pool = pctx.enter_context(tc.tile_pool(name="xt", bufs=2))
        wpool = pctx.enter_context(tc.tile_pool(name="wpool", bufs=1))
        spsum = pctx.enter_context(tc.tile_pool(name="spsum", bufs=1, space="PSUM"))

        wq_sb = [wpool.tile([P, HCOLS], BF16, tag=f"wq{i}", name=f"wq{i}") for i in range(KC)]
        wk_sb = [wpool.tile([P, HCOLS], BF16, tag=f"wk{i}", name=f"wk{i}") for i in range(KC)]
        wv_sb = [wpool.tile([P, HCOLS], BF16, tag=f"wv{i}", name=f"wv{i}") for i in range(KC)]

        # DMA emission order = need order: v path first. Early loads
        # round-robin across the two HWDGE queues (SP + Activation).
        _dma_rr = [0]

        def hw_dma(out_ap, in_ap):
            eng = nc.sync if _dma_rr[0] % 2 == 0 else nc.scalar
            _dma_rr[0] += 1
            eng.dma_start(out=out_ap, in_=in_ap)

        nc.gpsimd.dma_start(out=bq_sb[:], in_=bq[:])
        nc.gpsimd.dma_start(out=bk_sb[:], in_=bk[:])
        nc.gpsimd.dma_start(out=bv_sb[:], in_=bv[:])
        make_identity(nc, identity)
        make_identity(nc, identity_f)

        def load_xT(x_ap):
            tiles = []
            for c in range(KC):
                t = xt_pool.tile([P, L], BF16, tag=f"xT{c}", name=f"xT{c}")
                if c % 3 == 2:
                    nc.gpsimd.dma_start(out=t[:], in_=x_ap[c * P:(c + 1) * P, :])
                else:
                    hw_dma(t[:], x_ap[c * P:(c + 1) * P, :])
                tiles.append(t)
            return tiles

        # zero the padding halves of the per-head kT tiles once
        for h in range(HPC):
            pad = slice(DK, P) if h % 2 == 0 else slice(0, DK)
            nc.vector.memset(kt[h][pad, :], 0.0)

        # ---- v projection (all of v is needed by the first attention head,
        # so it runs up front; natural layout with ones column, b_v deferred
        # to the ctxT eviction) ----
        for i in range(KC):
            hw_dma(wv_sb[i][:], wv[i * P:(i + 1) * P, :])
        xTv = load_xT(xv)
        for i in range(KC):
            hw_dma(wk_sb[i][:], wk[i * P:(i + 1) * P, :])
        # upfront projection groups get their own triple-buffered PSUM pool so
        # the eviction of group n doesn't stall group n+1's matmuls
        upctx = ExitStack()
        ppsum = upctx.enter_context(tc.tile_pool(name="ppsum", bufs=3, space="PSUM"))
        for lc in range(LC):
            ps = ppsum.tile([P, NB], F32, tag="uproj", name="uproj_ps")
            for kc in range(KC):
                nc.tensor.matmul(
                    ps[:],
                    lhsT=xTv[kc][:, lc * P:(lc + 1) * P],
                    rhs=wv_sb[kc][:],
                    start=(kc == 0), stop=(kc == KC - 1),
                )
            ps3 = ps.rearrange("p (h d) -> p h d", d=DV)
            vt3 = vt[lc].rearrange("p (h w) -> p h w", w=VW)
            nc.vector.tensor_copy(out=vt3[:, :, 0:DV], in_=ps3[:])
            nc.vector.memset(vt3[:, :, DV:VW], 1.0)

        # ---- q/k projection groups. Only i=0 runs up front; i=1..3 are
        # sprinkled one matmul per attention k-iteration into the ACT-bound
        # attention loop (PE has slack there), paced to finish before the
        # head pair that needs them. ----
        xTk = load_xT(xk)
        for i in range(KC):
            hw_dma(wq_sb[i][:], wq[i * P:(i + 1) * P, :])
        xTq = load_xT(xq)

        def k_group(i, nb, pool=None):
            pool = pool or spsum
            ps = pool.tile([P, NB], F32, tag="proj", name="proj_ps")
            sl = slice(nb * NB, (nb + 1) * NB)

            def mm(kc):
                nc.tensor.matmul(
                    ps[:],
                    lhsT=wk_sb[kc][:, i * P:(i + 1) * P],
                    rhs=xTk[kc][:, sl],
                    start=(kc == 0), stop=(kc == KC - 1),
                )

            def evict():
                nc.vector.tensor_scalar_add(
                    kt[2 * i][0:DK, sl], ps[0:DK, :], bk_sb[0:DK, i:i + 1]
                )
                nc.vector.tensor_scalar_add(
                    kt[2 * i + 1][DK:P, sl], ps[DK:P, :], bk_sb[DK:P, i:i + 1]
                )

            return [lambda kc=kc: mm(kc) for kc in range(KC)] + [evict]

        def q_group(i, nb, pool=None):
            pool = pool or spsum
            ps = pool.tile([P, NB], F32, tag="proj", name="proj_ps")
            sl = slice(nb * NB, (nb + 1) * NB)

            def mm(kc):
                nc.tensor.matmul(
                    ps[:],
                    lhsT=wq_sb[kc][:, i * P:(i + 1) * P],
                    rhs=xTq[kc][:, sl],
                    start=(kc == 0), stop=(kc == KC - 1),
                )

            def evict():
                nc.vector.tensor_scalar_add(
                    qt[i][:, sl], ps[:], bq_sb[:, i:i + 1]
                )

            return [lambda kc=kc: mm(kc) for kc in range(KC)] + [evict]

        for nb in range(L // NB):
            for step in k_group(0, nb, ppsum):
                step()
        for nb in range(L // NB):
            for step in q_group(0, nb, ppsum):
                step()
        # run a few i=1 groups upfront so the sprinkle pace of one matmul per
        # attention iteration meets every head pair's deadline
        for kind, i, nb in (("k", 1, 0), ("k", 1, 1), ("q", 1, 0)):
            for step in (k_group if kind == "k" else q_group)(i, nb, ppsum):
                step()
        upctx.close()

        # lazily-expanded queue of sprinkled projection work
        upfront = {("k", 1, 0), ("k", 1, 1), ("q", 1, 0)}
        pending_groups = []
        for i in range(1, 4):
            for nb in range(L // NB):
                if ("k", i, nb) not in upfront:
                    pending_groups.append(("k", i, nb))
            for nb in range(L // NB):
                if ("q", i, nb) not in upfront:
                    pending_groups.append(("q", i, nb))
        pending_steps = []

        def sprinkle(n):
            for _ in range(n):
                if not pending_steps and pending_groups:
                    kind, i, nb = pending_groups.pop(0)
                    pending_steps.extend(k_group(i, nb) if kind == "k" else q_group(i, nb))
                if pending_steps:
                    step = pending_steps.pop(0)
                    step()
                    # run the eviction together with the last matmul
                    if len(pending_steps) == 1:
                        pending_steps.pop(0)()

        # ---- attention ----
        # Per (head, q-block): S^T = kT.T @ qT into PSUM [128k, QB] (dbuf),
        # P^T = exp(S^T/8) to SBUF bf16, then PV with v~ STATIONARY:
        # ctx~T[65, QB] += v~[k].T @ P^T  (row 64 = softmax denominator).
        # ctx~T is evicted to SBUF f32, PE-transposed back to natural
        # [128q, 65] chunks and normalized with per-partition reciprocals.
        with ExitStack() as actx:
            s_pool = actx.enter_context(tc.tile_pool(name="spool", bufs=2, space="PSUM"))
            c_pool = actx.enter_context(tc.tile_pool(name="cpool", bufs=1, space="PSUM"))
            tb_pool = actx.enter_context(tc.tile_pool(name="tbpool", bufs=1, space="PSUM"))
            pt_pool = actx.enter_context(tc.tile_pool(name="ptpool", bufs=3))
            cu_pool = actx.enter_context(tc.tile_pool(name="cupool", bufs=2))
            rv_pool = actx.enter_context(tc.tile_pool(name="rvpool", bufs=4))

            # Normalization of block b (transpose-back + reciprocal + scale) is
            # deferred and drip-fed into block b+1's k-loop so the serial
            # tb-chain never blocks the PE queue at a block boundary.
            deferred_norm = []

            def make_norm_step(cu_sb, h, qb, qc):
                def step():
                    tb = tb_pool.tile([P, VW], F32, tag="tb", name="tb")
                    nc.tensor.transpose(
                        tb[:], cu_sb[:, qc * P:(qc + 1) * P], identity_f[0:VW, 0:VW]
                    )
                    rinv = rv_pool.tile([P, 1], F32, tag="rinv", name="rinv")
                    nc.vector.reciprocal(rinv[:], tb[:, DV:VW])
                    lc = qb * (QB // P) + qc
                    nc.vector.tensor_scalar_mul(
                        ctx_t[lc][:, h * DV:(h + 1) * DV],
                        tb[:, 0:DV],
                        rinv[:],
                    )
                return step

            def emit_S(bh, bqoff, k):
                s = s_pool.tile([P, QB], F32, tag="s", name="s_ps")
                for half in range(QB // NB):
                    nc.tensor.matmul(
                        s[:, half * NB:(half + 1) * NB],
                        lhsT=kt[bh][:, k * P:(k + 1) * P],
                        rhs=qt[bh // 2][:, bqoff + half * NB:bqoff + (half + 1) * NB],
                        start=True, stop=True,
                    )
                return s

            blocks = [(h, qb) for h in range(HPC) for qb in range(NQB)]
            deferred_ctxT = []
            tail = {}

            def make_ctxT_step(i2, lc2):
                def step():
                    # same (space, bytes) as the sprinkle-projection PSUM tag,
                    # so this reuses that slot instead of needing a 9th bank
                    tp = spsum.tile([P, 1024], BF16, tag="proj", name="tp")[:, 0:2 * P]
                    for j in range(2):
                        nc.tensor.transpose(
                            tp[:, j * P:(j + 1) * P],
                            ctx_t[lc2 + j][:, i2 * P:(i2 + 1) * P],
                            identity[:],
                        )
                    nc.vector.tensor_scalar_add(
                        tail["ctxT"][i2][:, lc2 * P:(lc2 + 2) * P], tp[:],
                        bv_sb[:, i2:i2 + 1],
                    )
                return step

            s_next = emit_S(blocks[0][0], blocks[0][1] * QB, 0)
            for bi, (h, qb) in enumerate(blocks):
                if bi == 10:
                    # the sprinkled projections have drained: reuse the xT
                    # tile slots (same tag/shape) for ctxT and the W_o tiles,
                    # so the ctx transposes for head pairs 0..2 and the W_o
                    # load overlap the remaining attention blocks.
                    sprinkle(1 << 30)
                    ctxT = [xt_pool.tile([P, L], BF16, tag=f"xT{i2}", name=f"ctxT{i2}")
                            for i2 in range(4)]
                    wo_sb = [xt_pool.tile([P, L], BF16, tag=f"xT{4 + i2}", name=f"wo{i2}")
                             for i2 in range(4)]
                    for i2 in range(4):
                        nc.sync.dma_start(out=wo_sb[i2][:, 0:D], in_=wo[i2 * P:(i2 + 1) * P, :])
                    tail.update(wo_sb=wo_sb, ctxT=ctxT)
                    for i2 in range(3):
                        for lc2 in range(0, LC, 2):
                            deferred_ctxT.append(make_ctxT_step(i2, lc2))
                qoff = qb * QB
                cu = c_pool.tile([VW, QB], F32, tag="cu", name="cu")
                for k in range(LC):
                    s_cur = s_next
                    pt = pt_pool.tile([P, QB], BF16, tag="pt", name="pt")
                    nc.scalar.activation(pt[:], s_cur[:], EXP, scale=0.125)
                    # emit the next S immediately (the next block's S(0) at the
                    # block boundary) so the ACT engine is never starved
                    if k + 1 < LC:
                        s_next = emit_S(h, qoff, k + 1)
                    elif bi + 1 < len(blocks):
                        h2, qb2 = blocks[bi + 1]
                        s_next = emit_S(h2, qb2 * QB, 0)
                    if deferred_norm:
                        deferred_norm.pop(0)()
                    sprinkle(2 if k % 2 == 0 else 1)
                    if k % 2 == 1 and deferred_ctxT:
                        deferred_ctxT.pop(0)()
                    for half in range(QB // NB):
                        nc.tensor.matmul(
                            cu[:, half * NB:(half + 1) * NB],
                            lhsT=vt[k][:, h * VW:(h + 1) * VW],
                            rhs=pt[:, half * NB:(half + 1) * NB],
                            start=(k == 0), stop=(k == LC - 1),
                        )
                # evict unnormalized ctx~T (f32, keeps denom precision)
                cu_sb = cu_pool.tile([VW, QB], F32, tag="cusb", name="cu_sb")
                nc.vector.tensor_copy(out=cu_sb[:], in_=cu[:])
                for qc in range(QB // P):
                    deferred_norm.append(make_norm_step(cu_sb, h, qb, qc))
            # ---- tail, still inside the attention pool scope so dead PSUM
            # slots can be reused by tag. Interleave the last block's norms,
            # the head-pair-3 ctx transposes, and the output projection: the
            # outproj hc0-2 matmuls have no dependency on the drain chains and
            # fill the PE while they complete.
            for step in deferred_ctxT:
                step()
            sprinkle(1 << 30)  # safety: drain anything left
            osb = actx.enter_context(tc.tile_pool(name="osb", bufs=3))
            wo_sb, ctxT = tail["wo_sb"], tail["ctxT"]
            assert len(deferred_norm) == 8  # final block (h=7, qb=1): lc 8..15
            _slot = [0]

            def out_group(lc, half):
                # rotate across the dead proj/S/cu PSUM slots (same byte size)
                s = _slot[0] % 4
                _slot[0] += 1
                if s == 0:
                    ops = spsum.tile([P, NB], F32, tag="proj", name="ops")
                elif s == 3:
                    ops = c_pool.tile([P, 2 * NB], F32, tag="cu", name="ops")[:, 0:NB]
                else:
                    ops = s_pool.tile([P, 2 * NB], F32, tag="s", name="ops")[:, 0:NB]
                for hc in range(4):
                    nc.tensor.matmul(
                        ops[:],
                        lhsT=ctxT[hc][:, lc * P:(lc + 1) * P],
                        rhs=wo_sb[hc][:, half * NB:(half + 1) * NB],
                        start=(hc == 0), stop=(hc == 3),
                    )
                o_sb = osb.tile([P, NB], F32, tag="osb", name="o_sb")
                if half == 0:
                    nc.vector.tensor_copy(out=o_sb[:], in_=ops[:])
                else:
                    nc.scalar.copy(o_sb[:], ops[:])
                nc.sync.dma_start(
                    out=out[lc * P:(lc + 1) * P, half * NB:(half + 1) * NB],
                    in_=o_sb[:],
                )

            for lc2 in range(0, LC, 2):
                if lc2 >= 8:
                    deferred_norm.pop(0)()
                    deferred_norm.pop(0)()
                make_ctxT_step(3, lc2)()
                for lc in (lc2, lc2 + 1):
                    for half in range(D // NB):
                        out_group(lc, half)

        pctx.close()

    nc.compile()
    return nc


def _prep_inputs(query, key, value, W_q, b_q, W_k, b_k, W_v, b_v, W_o, b_o):
    bf = ml_dtypes.bfloat16
    # cast to bf16 and pre-transpose each batch to [D, L] on the host
    xq = np.ascontiguousarray(np.asarray(query, dtype=bf).transpose(0, 2, 1))
    xk = np.ascontiguousarray(np.asarray(key, dtype=bf).transpose(0, 2, 1))
    xv = np.ascontiguousarray(np.asarray(value, dtype=bf).transpose(0, 2, 1))

    def btile(b_slice):
        # [512] -> [128, 4] with tile[p, c] = b[c*128 + p]
        return np.ascontiguousarray(
            np.asarray(b_slice, np.float32).reshape(HCOLS // P, P).T
        )

    in_maps = []
    for c in range(8):
        b, hg = c // 2, c % 2
        sl = slice(hg * HCOLS, (hg + 1) * HCOLS)
        in_maps.append({
            "xq": xq[b], "xk": xk[b], "xv": xv[b],
            "wq": np.asarray(W_q[:, sl], dtype=bf),
            "wk": np.asarray(W_k[:, sl], dtype=bf),
            "wv": np.asarray(W_v[:, sl], dtype=bf),
            "wo": np.asarray(W_o[sl, :], dtype=bf),
            "bq": btile(b_q[sl]),
            "bk": btile(b_k[sl]),
            "bv": btile(b_v[sl]),
        })
    return in_maps


def kernel(query, key, value, W_q, b_q, W_k, b_k, W_v, b_v, W_o, b_o, **run_kwargs):
    if "nc" not in _CACHE:
        _CACHE["nc"] = build()
    nc = _CACHE["nc"]
    in_maps = _prep_inputs(query, key, value, W_q, b_q, W_k, b_k, W_v, b_v, W_o, b_o)
    res = bass_utils.run_bass_kernel_spmd(nc, in_maps, core_ids=list(range(8)), **run_kwargs)
    _CACHE["last_results"] = res
    out = np.empty((B, L, D), np.float32)
    bo = np.asarray(b_o, np.float32)
    for b in range(B):
        out[b] = res.results[2 * b]["out"] + res.results[2 * b + 1]["out"] + bo
    return out


# revision 39
# speedup vs baseline: 1.0106x; 1.0106x over previous
"""Multi-head attention Trainium2 kernel.

Problem: B=4, L=2048, D=1024, H=16 heads (dk=dv=64).
Sharding: 8 cores = 4 batches x 2 head-groups. Core c handles batch c//2 and
heads (c%2)*8 .. (c%2)*8+8 (column-parallel W_q/W_k/W_v, row-parallel W_o).
Host sums each batch's two partial outputs and adds b_o.

Per-core dataflow (all matmuls bf16 with f32 PSUM accumulation):
  xT    = host-pretransposed x, bf16            [D, L]   (q, k, v inputs)
  qT_i  = W_q_chunk.T @ xT  (+b_q)              [128=2 heads, L]
  kT_h  = same for k, zero-padded per head      [128, L]  (K=128 keeps HAM warm)
  v     = xT_chunk.T @ W_v (natural layout)     [L, 8*65] with ones column
  S^T   = kT_h.T @ qT                           [Lk, Lq]  scores transposed
  P^T   = exp(S^T / 8)   (ScalarE, no max-sub: scores are bounded ~|2.5|)
  ctx~T = v~.T @ P^T     (v stationary)         [65, Lq]; row 64 = softmax denom
  ctx   = PE-transpose-back(ctx~T) * (1/denom)  normalized, per-partition scalars
  ctxT  = PE-transpose(ctx) + b_v               [8*64, L]
  out   = ctxT.T @ W_o   (natural [L, D])       partial f32, summed on host

Scheduling: the i=1..3 q/k projection groups, the normalization chains, and
the head-pair 0..2 ctx transposes are drip-fed into the ACT-bound attention
k-loop (one op per exp) so the PE uses the ScalarE slack; S(0) of each next
block is hoisted across the block boundary to keep the exp stream dense.
"""

import numpy as np
import ml_dtypes
from contextlib import ExitStack

import concourse.bacc as bacc
import concourse.mybir as mybir
import concourse.tile as tile
from concourse import bass_utils
from concourse.masks import make_identity

P = 128
B, L, D = 4, 2048, 1024
H, DK, DV = 16, 64, 64
HPC = 8            # heads per core
HCOLS = HPC * DK   # 512 projection columns per core
KC = D // P        # 8 contraction chunks
LC = L // P        # 16 L chunks
NB = 512           # matmul free-dim block (one PSUM bank of f32)
QB = 1024          # attention q-block
NQB = L // QB      # 2
VW = DV + 1        # 65: v plus ones column
BF16 = mybir.dt.bfloat16
F32 = mybir.dt.float32

_CACHE = {}


def build(dump=False):
    nc = bacc.Bacc("TRN2", target_bir_lowering=False, debug=False)

    # x inputs arrive pre-transposed from the host: [D, L] contiguous
    xq = nc.dram_tensor("xq", (D, L), BF16, kind="ExternalInput").ap()
    xk = nc.dram_tensor("xk", (D, L), BF16, kind="ExternalInput").ap()
    xv = nc.dram_tensor("xv", (D, L), BF16, kind="ExternalInput").ap()
    wq = nc.dram_tensor("wq", (D, HCOLS), BF16, kind="ExternalInput").ap()
    wk = nc.dram_tensor("wk", (D, HCOLS), BF16, kind="ExternalInput").ap()
    wv = nc.dram_tensor("wv", (D, HCOLS), BF16, kind="ExternalInput").ap()
    wo = nc.dram_tensor("wo", (HCOLS, D), BF16, kind="ExternalInput").ap()
    bq = nc.dram_tensor("bq", (P, HCOLS // P), F32, kind="ExternalInput").ap()
    bk = nc.dram_tensor("bk", (P, HCOLS // P), F32, kind="ExternalInput").ap()
    bv = nc.dram_tensor("bv", (P, HCOLS // P), F32, kind="ExternalInput").ap()
    out = nc.dram_tensor("out", (L, D), F32, kind="ExternalOutput").ap()
    if dump:
        d_qt = nc.dram_tensor("d_qt", (4, P, L), BF16, kind="ExternalOutput").ap()
        d_kt = nc.dram_tensor("d_kt", (HPC, P, L), BF16, kind="ExternalOutput").ap()
        d_vt = nc.dram_tensor("d_vt", (LC, P, HPC * VW), BF16, kind="ExternalOutput").ap()
        d_ctx = nc.dram_tensor("d_ctx", (LC, P, HCOLS), BF16, kind="ExternalOutput").ap()
        d_ctxT = nc.dram_tensor("d_ctxT", (4, P, L), BF16, kind="ExternalOutput").ap()

    EXP = mybir.ActivationFunctionType.Exp

    with tile.TileContext(nc) as tc, ExitStack() as ctx:
        persist = ctx.enter_context(tc.tile_pool(name="persist", bufs=1))

        # ---- persistent tiles ----
        # qt tile i holds heads 2i (rows 0:64) and 2i+1 (rows 64:128).
        qt = [persist.tile([P, L], BF16, tag=f"qt{i}", name=f"qt{i}") for i in range(4)]
        # kt is stored per head, zero-padded to the full 128 partitions so the
        # S^T matmul contracts K=128: head 2i lives in rows 0:64 (rows 64:128
        # zero), head 2i+1 in rows 64:128 (rows 0:64 zero). Both heads then
        # share the same full-height qt tile as the moving operand, and K=128
        # keeps the PE HAM activity monitor warm (K=64 matmuls don't register
        # as PE-busy and leave the clock throttled at 1.2 GHz).
        kt = [persist.tile([P, L], BF16, tag=f"kt{h}", name=f"kt{h}") for h in range(HPC)]
        # v~ tiles: [128 k-positions, 8 heads * 65]; col h*65+64 = 1.0
        vt = [persist.tile([P, HPC * VW], BF16, tag=f"vt{i}", name=f"vt{i}") for i in range(LC)]
        # ctx natural [L, 512] as 16 row tiles
        ctx_t = [persist.tile([P, HCOLS], BF16, tag=f"ctx{i}", name=f"ctx{i}") for i in range(LC)]
        bq_sb = persist.tile([P, HCOLS // P], F32, tag="bq", name="bq_sb")
        bk_sb = persist.tile([P, HCOLS // P], F32, tag="bk", name="bk_sb")
        bv_sb = persist.tile([P, HCOLS // P], F32, tag="bv", name="bv_sb")
        identity = persist.tile([P, P], BF16, tag="ident", name="ident")
        identity_f = persist.tile([P, P], F32, tag="identf", name="identf")

        # ---- projection-phase pool (lives until the last sprinkled
        # projection group inside the attention loop finishes) ----
        pctx = ExitStack()
        xt_pool = pctx.enter_context(tc.tile_pool(name="xt", bufs=2))
        wpool = pctx.enter_context(tc.tile_pool(name="wpool", bufs=1))
        spsum = pctx.enter_context(tc.tile_pool(name="spsum", bufs=1, space="PSUM"))

        wq_sb = [wpool.tile([P, HCOLS], BF16, tag=f"wq{i}", name=f"wq{i}") for i in range(KC)]
        wk_sb = [wpool.tile([P, HCOLS], BF16, tag=f"wk{i}", name=f"wk{i}") for i in range(KC)]
        wv_sb = [wpool.tile([P, HCOLS], BF16, tag=f"wv{i}", name=f"wv{i}") for i in range(KC)]

        # DMA emission order = need order: v path first. Early loads
        # round-robin across the two HWDGE queues (SP + Activation).
        _dma_rr = [0]

        def hw_dma(out_ap, in_ap):
            eng = nc.sync if _dma_rr[0] % 2 == 0 else nc.scalar
            _dma_rr[0] += 1
            eng.dma_start(out=out_ap, in_=in_ap)

        nc.gpsimd.dma_start(out=bq_sb[:], in_=bq[:])
        nc.gpsimd.dma_start(out=bk_sb[:], in_=bk[:])
        nc.gpsimd.dma_start(out=bv_sb[:], in_=bv[:])
        make_identity(nc, identity)
        make_identity(nc, identity_f)

        def load_xT(x_ap):
            tiles = []
            for c in range(KC):
                t = xt_pool.tile([P, L], BF16, tag=f"xT{c}", name=f"xT{c}")
                hw_dma(t[:], x_ap[c * P:(c + 1) * P, :])
                tiles.append(t)
            return tiles

        # zero the padding halves of the per-head kT tiles once
        for h in range(HPC):
            pad = slice(DK, P) if h % 2 == 0 else slice(0, DK)
            nc.vector.memset(kt[h][pad, :], 0.0)

        # ---- v projection (all of v is needed by the first attention head,
        # so it runs up front; natural layout with ones column, b_v deferred
        # to the ctxT eviction) ----
        for i in range(KC):
            hw_dma(wv_sb[i][:], wv[i * P:(i + 1) * P, :])
        xTv = load_xT(xv)
        for i in range(KC):
            hw_dma(wk_sb[i][:], wk[i * P:(i + 1) * P, :])
        # upfront projection groups get their own triple-buffered PSUM pool so
        # the eviction of group n doesn't stall group n+1's matmuls
        upctx = ExitStack()
        ppsum = upctx.enter_context(tc.tile_pool(name="ppsum", bufs=3, space="PSUM"))
        for lc in range(LC):
            ps = ppsum.tile([P, NB], F32, tag="uproj", name="uproj_ps")
            for kc in range(KC):
                nc.tensor.matmul(
                    ps[:],
                    lhsT=xTv[kc][:, lc * P:(lc + 1) * P],
                    rhs=wv_sb[kc][:],
                    start=(kc == 0), stop=(kc == KC - 1),
                )
            ps3 = ps.rearrange("p (h d) -> p h d", d=DV)
            vt3 = vt[lc].rearrange("p (h w) -> p h w", w=VW)
            nc.vector.tensor_copy(out=vt3[:, :, 0:DV], in_=ps3[:])
            nc.vector.memset(vt3[:, :, DV:VW], 1.0)

        # ---- q/k projection groups. Only i=0 runs up front; i=1..3 are
        # sprinkled one matmul per attention k-iteration into the ACT-bound
        # attention loop (PE has slack there), paced to finish before the
        # head pair that needs them. ----
        xTk = load_xT(xk)
        for i in range(KC):
            hw_dma(wq_sb[i][:], wq[i * P:(i + 1) * P, :])
        xTq = load_xT(xq)

        def k_group(i, nb, pool=None):
            pool = pool or spsum
            ps = pool.tile([P, NB], F32, tag="proj", name="proj_ps")
            sl = slice(nb * NB, (nb + 1) * NB)

            def mm(kc):
                nc.tensor.matmul(
                    ps[:],
                    lhsT=wk_sb[kc][:, i * P:(i + 1) * P],
                    rhs=xTk[kc][:, sl],
                    start=(kc == 0), stop=(kc == KC - 1),
                )

            def evict():
                nc.vector.tensor_scalar_add(
                    kt[2 * i][0:DK, sl], ps[0:DK, :], bk_sb[0:DK, i:i + 1]
                )
                nc.vector.tensor_scalar_add(
                    kt[2 * i + 1][DK:P, sl], ps[DK:P, :], bk_sb[DK:P, i:i + 1]
                )

            return [lambda kc=kc: mm(kc) for kc in range(KC)] + [evict]

        def q_group(i, nb, pool=None):
            pool = pool or spsum
            ps = pool.tile([P, NB], F32, tag="proj", name="proj_ps")
            sl = slice(nb * NB, (nb + 1) * NB)

            def mm(kc):
                nc.tensor.matmul(
                    ps[:],
                    lhsT=wq_sb[kc][:, i * P:(i + 1) * P],
                    rhs=xTq[kc][:, sl],
                    start=(kc == 0), stop=(kc == KC - 1),
                )

            def evict():
                nc.vector.tensor_scalar_add(
                    qt[i][:, sl], ps[:], bq_sb[:, i:i + 1]
                )

            return [lambda kc=kc: mm(kc) for kc in range(KC)] + [evict]

        for nb in range(L // NB):
            for step in k_group(0, nb, ppsum):
                step()
        for nb in range(L // NB):
            for step in q_group(0, nb, ppsum):
                step()
        # run a few i=1 groups upfront so the sprinkle pace of one matmul per
        # attention iteration meets every head pair's deadline
        for kind, i, nb in (("k", 1, 0), ("k", 1, 1), ("q", 1, 0)):
            for step in (k_group if kind == "k" else q_group)(i, nb, ppsum):
                step()
        upctx.close()

        # lazily-expanded queue of sprinkled projection work
        upfront = {("k", 1, 0), ("k", 1, 1), ("q", 1, 0)}
        pending_groups = []
        for i in range(1, 4):
            for nb in range(L // NB):
                if ("k", i, nb) not in upfront:
                    pending_groups.append(("k", i, nb))
            for nb in range(L // NB):
                if ("q", i, nb) not in upfront:
                    pending_groups.append(("q", i, nb))
        pending_steps = []

        def sprinkle(n):
            for _ in range(n):
                if not pending_steps and pending_groups:
                    kind, i, nb = pending_groups.pop(0)
                    pending_steps.extend(k_group(i, nb) if kind == "k" else q_group(i, nb))
                if pending_steps:
                    step = pending_steps.pop(0)
                    step()
                    # run the eviction together with the last matmul
                    if len(pending_steps) == 1:
                        pending_steps.pop(0)()

        # ---- attention ----
        # Per (head, q-block): S^T = kT.T @ qT into PSUM [128k, QB] (dbuf),
        # P^T = exp(S^T/8) to SBUF bf16, then PV with v~ STATIONARY:
        # ctx~T[65, QB] += v~[k].T @ P^T  (row 64 = softmax denominator).
        # ctx~T is evicted to SBUF f32, PE-transposed back to natural
        # [128q, 65] chunks and normalized with per-partition reciprocals.
        with ExitStack() as actx:
            s_pool = actx.enter_context(tc.tile_pool(name="spool", bufs=2, space="PSUM"))
            c_pool = actx.enter_context(tc.tile_pool(name="cpool", bufs=1, space="PSUM"))
            tb_pool = actx.enter_context(tc.tile_pool(name="tbpool", bufs=1, space="PSUM"))
            pt_pool = actx.enter_context(tc.tile_pool(name="ptpool", bufs=3))
            cu_pool = actx.enter_context(tc.tile_pool(name="cupool", bufs=2))
            rv_pool = actx.enter_context(tc.tile_pool(name="rvpool", bufs=4))

            # Normalization of block b (transpose-back + reciprocal + scale) is
            # deferred and drip-fed into block b+1's k-loop so the serial
            # tb-chain never blocks the PE queue at a block boundary.
            deferred_norm = []

            def make_norm_step(cu_sb, h, qb, qc):
                def step():
                    tb = tb_pool.tile([P, VW], F32, tag="tb", name="tb")
                    nc.tensor.transpose(
                        tb[:], cu_sb[:, qc * P:(qc + 1) * P], identity_f[0:VW, 0:VW]
                    )
                    rinv = rv_pool.tile([P, 1], F32, tag="rinv", name="rinv")
                    nc.vector.reciprocal(rinv[:], tb[:, DV:VW])
                    lc = qb * (QB // P) + qc
                    nc.vector.tensor_scalar_mul(
                        ctx_t[lc][:, h * DV:(h + 1) * DV],
                        tb[:, 0:DV],
                        rinv[:],
                    )
                return step

            def emit_S(bh, bqoff, k):
                s = s_pool.tile([P, QB], F32, tag="s", name="s_ps")
                for half in range(QB // NB):
                    nc.tensor.matmul(
                        s[:, half * NB:(half + 1) * NB],
                        lhsT=kt[bh][:, k * P:(k + 1) * P],
                        rhs=qt[bh // 2][:, bqoff + half * NB:bqoff + (half + 1) * NB],
                        start=True, stop=True,
                    )
                return s

            blocks = [(h, qb) for h in range(HPC) for qb in range(NQB)]
            deferred_ctxT = []
            tail = {}

            def make_ctxT_step(i2, lc2):
                def step():
                    # same (space, bytes) as the sprinkle-projection PSUM tag,
                    # so this reuses that slot instead of needing a 9th bank
                    tp = spsum.tile([P, 1024], BF16, tag="proj", name="tp")[:, 0:2 * P]
                    for j in range(2):
                        nc.tensor.transpose(
                            tp[:, j * P:(j + 1) * P],
                            ctx_t[lc2 + j][:, i2 * P:(i2 + 1) * P],
                            identity[:],
                        )
                    nc.vector.tensor_scalar_add(
                        tail["ctxT"][i2][:, lc2 * P:(lc2 + 2) * P], tp[:],
                        bv_sb[:, i2:i2 + 1],
                    )
                return step

            s_next = emit_S(blocks[0][0], blocks[0][1] * QB, 0)
            for bi, (h, qb) in enumerate(blocks):
                if bi == 10:
                    # the sprinkled projections have drained: reuse the xT
                    # tile slots (same tag/shape) for ctxT and the W_o tiles,
                    # so the ctx transposes for head pairs 0..2 and the W_o
                    # load overlap the remaining attention blocks.
                    sprinkle(1 << 30)
                    ctxT = [xt_pool.tile([P, L], BF16, tag=f"xT{i2}", name=f"ctxT{i2}")
                            for i2 in range(4)]
                    wo_sb = [xt_pool.tile([P, L], BF16, tag=f"xT{4 + i2}", name=f"wo{i2}")
                             for i2 in range(4)]
                    for i2 in range(4):
                        nc.sync.dma_start(out=wo_sb[i2][:, 0:D], in_=wo[i2 * P:(i2 + 1) * P, :])
                    tail.update(wo_sb=wo_sb, ctxT=ctxT)
                    for i2 in range(3):
                        for lc2 in range(0, LC, 2):
                            deferred_ctxT.append(make_ctxT_step(i2, lc2))
                qoff = qb * QB
                cu = c_pool.tile([VW, QB], F32, tag="cu", name="cu")
                for k in range(LC):
                    s_cur = s_next
                    pt = pt_pool.tile([P, QB], BF16, tag="pt", name="pt")
                    nc.scalar.activation(pt[:], s_cur[:], EXP, scale=0.125)
                    # emit the next S immediately (the next block's S(0) at the
                    # block boundary) so the ACT engine is never starved
                    if k + 1 < LC:
                        s_next = emit_S(h, qoff, k + 1)
                    elif bi + 1 < len(blocks):
                        h2, qb2 = blocks[bi + 1]
                        s_next = emit_S(h2, qb2 * QB, 0)
                    if deferred_norm:
                        deferred_norm.pop(0)()
                    sprinkle(2 if k % 2 == 0 else 1)
                    if k % 2 == 1 and deferred_ctxT:
                        deferred_ctxT.pop(0)()
                    for half in range(QB // NB):
                        nc.tensor.matmul(
                            cu[:, half * NB:(half + 1) * NB],
                            lhsT=vt[k][:, h * VW:(h + 1) * VW],
                            rhs=pt[:, half * NB:(half + 1) * NB],
                            start=(k == 0), stop=(k == LC - 1),
                        )
                # evict unnormalized ctx~T (f32, keeps denom precision)
                cu_sb = cu_pool.tile([VW, QB], F32, tag="cusb", name="cu_sb")
                nc.vector.tensor_copy(out=cu_sb[:], in_=cu[:])
                for qc in range(QB // P):
                    deferred_norm.append(make_norm_step(cu_sb, h, qb, qc))
            # ---- tail, still inside the attention pool scope so dead PSUM
            # slots can be reused by tag. Interleave the last block's norms,
            # the head-pair-3 ctx transposes, and the output projection: the
            # outproj hc0-2 matmuls have no dependency on the drain chains and
            # fill the PE while they complete.
            for step in deferred_ctxT:
                step()
            sprinkle(1 << 30)  # safety: drain anything left
            osb = actx.enter_context(tc.tile_pool(name="osb", bufs=3))
            wo_sb, ctxT = tail["wo_sb"], tail["ctxT"]
            assert len(deferred_norm) == 8  # final block (h=7, qb=1): lc 8..15
            _slot = [0]

            def out_group(lc, half):
                # rotate across the dead proj/S/cu PSUM slots (same byte size)
                s = _slot[0] % 4
                _slot[0] += 1
                if s == 0:
                    ops = spsum.tile([P, NB], F32, tag="proj", name="ops")
                elif s == 3:
                    ops = c_pool.tile([P, 2 * NB], F32, tag="cu", name="ops")[:, 0:NB]
                else:
                    ops = s_pool.tile([P, 2 * NB], F32, tag="s", name="ops")[:, 0:NB]
                for hc in range(4):
                    nc.tensor.matmul(
                        ops[:],
                        lhsT=ctxT[hc][:, lc * P:(lc + 1) * P],
                        rhs=wo_sb[hc][:, half * NB:(half + 1) * NB],
                        start=(hc == 0), stop=(hc == 3),
                    )
                o_sb = osb.tile([P, NB], F32, tag="osb", name="o_sb")
                if half == 0:
                    nc.vector.tensor_copy(out=o_sb[:], in_=ops[:])
                else:
                    nc.scalar.copy(o_sb[:], ops[:])
                nc.sync.dma_start(
                    out=out[lc * P:(lc + 1) * P, half * NB:(half + 1) * NB],
                    in_=o_sb[:],
                )

            for lc2 in range(0, LC, 2):
                if lc2 >= 8:
                    deferred_norm.pop(0)()
                    deferred_norm.pop(0)()
                make_ctxT_step(3, lc2)()
                for lc in (lc2, lc2 + 1):
                    for half in range(D // NB):
                        out_group(lc, half)

        pctx.close()

    nc.compile()
    return nc


def _prep_inputs(query, key, value, W_q, b_q, W_k, b_k, W_v, b_v, W_o, b_o):
    bf = ml_dtypes.bfloat16
    # cast to bf16 and pre-transpose each batch to [D, L] on the host
    xq = np.ascontiguousarray(np.asarray(query, dtype=bf).transpose(0, 2, 1))
    xk = np.ascontiguousarray(np.asarray(key, dtype=bf).transpose(0, 2, 1))
    xv = np.ascontiguousarray(np.asarray(value, dtype=bf).transpose(0, 2, 1))

    def btile(b_slice):
        # [512] -> [128, 4] with tile[p, c] = b[c*128 + p]
        return np.ascontiguousarray(
            np.asarray(b_slice, np.float32).reshape(HCOLS // P, P).T
        )

    in_maps = []
    for c in range(8):
        b, hg = c // 2, c % 2
        sl = slice(hg * HCOLS, (hg + 1) * HCOLS)
        in_maps.append({
            "xq": xq[b], "xk": xk[b], "xv": xv[b],
            "wq": np.asarray(W_q[:, sl], dtype=bf),
            "wk": np.asarray(W_k[:, sl], dtype=bf),
            "wv": np.asarray(W_v[:, sl], dtype=bf),
            "wo": np.asarray(W_o[sl, :], dtype=bf),
            "bq": btile(b_q[sl]),
            "bk": btile(b_k[sl]),
            "bv": btile(b_v[sl]),
        })
    return in_maps


def kernel(query, key, value, W_q, b_q, W_k, b_k, W_v, b_v, W_o, b_o, **run_kwargs):
    if "nc" not in _CACHE:
        _CACHE["nc"] = build()
    nc = _CACHE["nc"]
    in_maps = _prep_inputs(query, key, value, W_q, b_q, W_k, b_k, W_v, b_v, W_o, b_o)
    res = bass_utils.run_bass_kernel_spmd(nc, in_maps, core_ids=list(range(8)), **run_kwargs)
    _CACHE["last_results"] = res
    out = np.empty((B, L, D), np.float32)
    bo = np.asarray(b_o, np.float32)
    for b in range(B):
        out[b] = res.results[2 * b]["out"] + res.results[2 * b + 1]["out"] + bo
    return out
